# revision 19
# baseline (speedup 1.0000x reference)
"""LoRA row-parallel linear on 8 TRN2 NeuronCores.

Problem: y = x @ W^T + delta, where per-token LoRA delta[t] = B[s] @ (A[s] @ x[t]),
s = token_to_slot[t] (8 adapters, rank 16, scaling baked into B).

Strategy: token data-parallel across the 8 cores (T=8192 -> 1024 tokens/core).
No collectives needed; each core computes its token block fully, in transposed
output space (y^T, un-transposed on the host):
  u^T   = A_all @ x_shard^T          (128 x T_SH; A_all = all 8 adapters stacked)
  uM^T  = u^T * mask^T               (one-hot select of each token's adapter)
  y^T   = W @ x^T + B_all^T @ uM^T   (PSUM accumulation per 128x512 block)

Precision/speed split of the base contraction (D_IN=4096 = 32 k-tiles):
  k-tiles 0..27  -> bf16 matmuls (1 col/cycle).
  k-tiles 28..31 -> fp8 e4m3 with perf_mode=DoubleRow: 2 k-tiles packed per
    instruction (pair dim in both operands), ~2x col rate. W/A are scaled x8
    and x by 1/8 on the host so products need no descaling and operands sit
    mostly in e4m3's normal range.
The u-pass runs entirely in fp8 DoubleRow (16 pair-groups, 32 MMs instead of
64): its error only reaches y through the rank-16 delta (~0.1 rms vs y scale
~7), so fp8 there costs <1e-3. Measured end-to-end max rel err ~1.5e-2
(gate 2e-2): bf16-only is 2.5e-3, the 4 fp8 base k-tiles add ~1.4e-2
(verified numerically against the exact inputs; even with fp8 denormals
flushed the bound is ~1.7e-2).

Schedule (per core): ob0's 28-k-tile d-loop runs FIRST, paced by the x^T
shard streaming in; the fp8 u-pass follows, then ob0's LoRA delta via a
separate accumulation + DVE add. Remaining obs append the 2 DoubleRow MMs and
the fused delta MM as accumulation steps 29..31. PSUM->SBUF drains alternate
Vector/Scalar. DMA issue-queue layout: Sync owns the W/x streams exclusively
(12-deep W pool so W never arrives late); Scalar issues everything else -
the LoRA/fp8 tensors (gated behind a dummy dependency on a mid-shard x tile
so their transfers don't steal HBM bandwidth from phase 1) and all output
DMAs (gpsimd DGE would add ~8us of epilogue DRAIN, so it is not used).

Host prep: transposes x/W/A to put the contraction dim on partitions, casts
to bf16, packs the fp8 pair-layout tensors, builds the one-hot mask from
token_to_slot. Device does all the FLOPs.
"""

import numpy as np
import ml_dtypes

from concourse import bacc, tile, mybir
from concourse.bass_utils import run_bass_kernel_spmd
import concourse.bass_utils as _bu

# Disable S3 artifact upload in the trace path (no credentials in this container).
_bu.upload_artifacts = lambda tmpdir: "local://" + tmpdir

N_CORES = 8
T = 8192
D_IN = 4096
D_OUT = 4096
L = 8          # max adapters
R = 16         # max rank
LR = L * R     # 128 = stacked adapter dim
T_SH = T // N_CORES          # 1024 tokens per core
KT = D_IN // 128             # 32 contraction tiles
KB = 28                      # base k-tiles done in bf16
GT = KT // 2                 # 16 fp8 pair-groups over the full contraction
GB = KB // 2                 # pair-groups 14,15 serve the base matmul tail
OB = D_OUT // 512            # 8 output-column superblocks
NO = 4                       # 128-wide output blocks per superblock
NT = T_SH // 512             # 2 token blocks (moving dim)

F32 = mybir.dt.float32
BF16 = mybir.dt.bfloat16
F8 = mybir.dt.float8e4
DR = mybir.MatmulPerfMode.DoubleRow

_CACHED_NC = None


def _build():
    nc = bacc.Bacc("TRN2", target_bir_lowering=False, debug=False)

    xT_d = nc.dram_tensor("xT", [KB * 128, T_SH], BF16, kind="ExternalInput")
    wT_d = nc.dram_tensor("wT", [D_IN, D_OUT], BF16, kind="ExternalInput")
    # fp8 pair-packed operands: row g*128+p holds k-rows (g*256 + p) as
    # pair-half i=0 and (g*256 + 128 + p) as i=1; x is pre-scaled by 1/8,
    # W/A tails by 8.
    xf8_d = nc.dram_tensor("xf8", [128, GT * 2048], F8, kind="ExternalInput")
    wf8_d = nc.dram_tensor("wf8", [(GT - GB) * 128, OB * 1024], F8,
                           kind="ExternalInput")
    af8_d = nc.dram_tensor("af8", [128, GT * 2 * LR], F8, kind="ExternalInput")
    bC_d = nc.dram_tensor("bC", [LR, D_OUT], BF16, kind="ExternalInput")
    mT_d = nc.dram_tensor("maskT", [LR, T_SH], BF16, kind="ExternalInput")
    yT_d = nc.dram_tensor("yT", [D_OUT, T_SH], BF16, kind="ExternalOutput")

    with tile.TileContext(nc) as tc:
        with (
            tc.tile_pool(name="resident", bufs=1) as rpool,
            tc.tile_pool(name="wstream", bufs=9) as wpool,
            tc.tile_pool(name="f8stream", bufs=4) as f8pool,
            tc.tile_pool(name="yout", bufs=12) as ypool,
            tc.tile_pool(name="psum", bufs=8, space="PSUM") as psum,
        ):
            # --- x shard + W superblock 0 interleaved on Sync; everything
            # --- else on Scalar's queue.
            xts = []
            wts0 = []
            for d in range(KB):
                wt = wpool.tile([128, 512], BF16, tag="wt", name=f"wt0_{d}")
                nc.sync.dma_start(wt[:], wT_d[d * 128:(d + 1) * 128, 0:512])
                wts0.append(wt)
                xt = rpool.tile([128, T_SH], BF16, tag=f"xt{d}")
                nc.sync.dma_start(xt[:], xT_d[d * 128:(d + 1) * 128, :])
                xts.append(xt)

            # Two-stage gating of the non-stream loads so their transfers
            # don't compete with phase 1's x/W streams: the base fp8 tail
            # tiles (needed ~52us) go behind a gate that opens ~20us; the
            # u-pass operands (needed only after ob1's base pass, ~105us)
            # behind a gate that opens when the x shard finishes (~52us).
            gate = rpool.tile([128, 2], BF16, tag="gate")
            nc.scalar.copy(gate[0:1, 0:2], xts[12][0:1, 0:2])
            xf8all = rpool.tile([128, GT * 2048], F8, tag="xf8all")
            # base-matmul tail groups (GB, GB+1) land first, behind gate 1
            nc.scalar.dma_start(xf8all[:, GB * 2048:(GB + 2) * 2048],
                                xf8_d[:, GB * 2048:(GB + 2) * 2048])
            gate2 = rpool.tile([128, 2], BF16, tag="gate2")
            nc.scalar.copy(gate2[0:1, 0:2], xts[KB - 1][0:1, 0:2])
            nc.scalar.dma_start(xf8all[:, 0:(GB // 2) * 2048],
                                xf8_d[:, 0:(GB // 2) * 2048])
            nc.scalar.dma_start(xf8all[:, (GB // 2) * 2048:GB * 2048],
                                xf8_d[:, (GB // 2) * 2048:GB * 2048])
            # af8 SBUF layout is strip-padded (pair halves at +0 and +512 of
            # each group's 1024-col block) so the DoubleRow weight AP has the
            # same pair-stride-512 shape as wf8 (stride 128 faults the PE);
            # one 4D strided DMA fills both strips from the compact tensor.
            af8all = rpool.tile([128, GT * 1024], F8, tag="af8all")
            af8_dst = (af8all[:, :]
                       .rearrange("p (g c) -> p g c", g=GT)
                       .rearrange("p g (i c) -> p g i c", i=2)[:, :, :, 0:LR])
            af8_src = af8_d[:, :].rearrange("p (g i c) -> p g i c", g=GT, i=2)
            nc.scalar.dma_start(af8_dst, af8_src)
            bc = rpool.tile([LR, D_OUT], BF16, tag="bc")
            nc.scalar.dma_start(bc[:], bC_d[:])
            mask = rpool.tile([LR, T_SH], BF16, tag="mask")
            nc.scalar.dma_start(mask[:], mT_d[:])
            uTms = [rpool.tile([LR, 512], BF16, tag=f"uTm{ub}", name=f"uTm{ub}")
                    for ub in range(NT)]

            def wf8_tiles(ob):
                tiles = []
                for g in range(2):
                    wf = f8pool.tile([128, 1024], F8, tag="wf8",
                                     name=f"wf8_{ob}_{g}")
                    nc.sync.dma_start(
                        wf[:], wf8_d[g * 128:(g + 1) * 128,
                                     ob * 1024:(ob + 1) * 1024])
                    tiles.append(wf)
                return tiles

            def xf8_rhs(g, t):
                base = g * 2048 + t * 1024
                return xf8all[:, base:base + 1024].rearrange(
                    "p (i c) -> p i c", i=2)

            def dr_matmul(ptile, wf, g, o, t, start, stop):
                lhs = wf[:, :].rearrange("p (i c) -> p i c", i=2)[
                    :, :, o * 128:(o + 1) * 128]
                nc.tensor.matmul(ptile[:], lhs, xf8_rhs(g, t), start=start,
                                 stop=stop, perf_mode=DR, skip_group_check=True)

            # --- phase 1: ob0 d-loop (base matmul only, no delta) --------------
            # psum tile (o, t) = y^T[o-block of 128, t-block of 512]
            wf80 = wf8_tiles(0)
            pys0 = [[psum.tile([128, 512], F32, tag="acc", name=f"py0_{o}_{t}")
                     for t in range(NT)] for o in range(NO)]
            yo0s = {}
            for d in range(KB):
                for o in range(NO):
                    lw = wts0[d][:, o * 128:(o + 1) * 128]
                    for t in range(NT):
                        nc.tensor.matmul(
                            pys0[o][t][:], lw, xts[d][:, t * 512:(t + 1) * 512],
                            start=(d == 0), stop=False, skip_group_check=True,
                        )
            for o in range(NO):
                for t in range(NT):
                    for g in range(2):
                        dr_matmul(pys0[o][t], wf80[g], GB + g, o, t,
                                  start=False, stop=(g == 1))
                    yo0 = rpool.tile([128, 512], BF16, tag=f"yo0_{o}_{t}",
                                     name=f"yo0_{o}_{t}")
                    if (o + t) % 2 == 0:
                        nc.vector.tensor_copy(yo0[:], pys0[o][t][:])
                    else:
                        nc.scalar.copy(yo0[:], pys0[o][t][:])
                    yo0s[o, t] = yo0

            # --- phase 1b: ob1 base pass (W-paced; the gated u-pass operand
            # --- streams land during it), results staged like ob0's --------
            yo1s = {}
            pys1 = [[psum.tile([128, 512], F32, tag="acc", name=f"py1_{o}_{t}")
                     for t in range(NT)] for o in range(NO)]
            wf81 = None
            for d in range(KB):
                wt = wpool.tile([128, 512], BF16, tag="wt", name=f"wt1_{d}")
                nc.sync.dma_start(
                    wt[:], wT_d[d * 128:(d + 1) * 128, 512:1024])
                if d == 0:
                    wf81 = wf8_tiles(1)
                for o in range(NO):
                    lw = wt[:, o * 128:(o + 1) * 128]
                    for t in range(NT):
                        nc.tensor.matmul(
                            pys1[o][t][:], lw, xts[d][:, t * 512:(t + 1) * 512],
                            start=(d == 0), stop=False, skip_group_check=True,
                        )
            for o in range(NO):
                for t in range(NT):
                    for g in range(2):
                        dr_matmul(pys1[o][t], wf81[g], GB + g, o, t,
                                  start=False, stop=(g == 1))
                    yo1 = rpool.tile([128, 512], BF16, tag=f"yo1_{o}_{t}",
                                     name=f"yo1_{o}_{t}")
                    if (o + t) % 2 == 0:
                        nc.vector.tensor_copy(yo1[:], pys1[o][t][:])
                    else:
                        nc.scalar.copy(yo1[:], pys1[o][t][:])
                    yo1s[o, t] = yo1

            # --- phase 2: fp8 DoubleRow u-pass ---------------------------------
            for ub in range(NT):
                pu = psum.tile([128, 512], F32, tag="acc", name=f"pu{ub}")
                for g in range(GT):
                    lhs = af8all[:, g * 1024:(g + 1) * 1024].rearrange(
                        "p (i c) -> p i c", i=2)[:, :, 0:LR]
                    nc.tensor.matmul(pu[:], lhs, xf8_rhs(g, ub),
                                     start=(g == 0), stop=(g == GT - 1),
                                     perf_mode=DR, skip_group_check=True)
                sl = slice(ub * 512, (ub + 1) * 512)
                nc.vector.tensor_mul(uTms[ub][:], pu[:], mask[:, sl])

            # --- phase 3: ob0/ob1 deltas + writeback ---------------------------
            # pd banks are freed by quick V/S copies (so the next superblock's
            # accumulation groups aren't starved of PSUM); the adds run on
            # GpSimd (SBUF-only) off the critical path.
            for ob, staged in ((0, yo0s), (1, yo1s)):
                for o in range(NO):
                    og = ob * 512 + o * 128
                    for t in range(NT):
                        pd = psum.tile([128, 512], F32, tag="acc",
                                       name=f"pd{ob}_{o}_{t}")
                        nc.tensor.matmul(
                            pd[:], bc[:, og:og + 128], uTms[t][:],
                            start=True, stop=True, skip_group_check=True,
                        )
                        tmp = ypool.tile([128, 512], BF16, tag="dtmp",
                                         name=f"dt{ob}_{o}_{t}")
                        if (o + t) % 2 == 0:
                            nc.vector.tensor_copy(tmp[:], pd[:])
                        else:
                            nc.scalar.copy(tmp[:], pd[:])
                        yo = ypool.tile([128, 512], BF16, tag="yo",
                                        name=f"yod{ob}_{o}_{t}")
                        nc.gpsimd.tensor_add(yo[:], staged[o, t][:], tmp[:])
                        nc.sync.dma_start(
                            yT_d[og:og + 128, t * 512:(t + 1) * 512], yo[:])

            # --- phase 4: ob2..7, fp8 tail + fused delta -----------------------
            for ob in range(2, OB):
                pys = [[psum.tile([128, 512], F32, tag="acc", name=f"py{ob}_{o}_{t}")
                        for t in range(NT)] for o in range(NO)]
                wf8t = None
                for d in range(KB):
                    wt = wpool.tile([128, 512], BF16, tag="wt", name=f"wt{ob}_{d}")
                    nc.sync.dma_start(
                        wt[:],
                        wT_d[d * 128:(d + 1) * 128, ob * 512:(ob + 1) * 512])
                    if d == 0:
                        wf8t = wf8_tiles(ob)
                    for o in range(NO):
                        lw = wt[:, o * 128:(o + 1) * 128]
                        for t in range(NT):
                            nc.tensor.matmul(
                                pys[o][t][:], lw, xts[d][:, t * 512:(t + 1) * 512],
                                start=(d == 0), stop=False, skip_group_check=True,
                            )
                for o in range(NO):
                    og = ob * 512 + o * 128
                    for t in range(NT):
                        for g in range(2):
                            dr_matmul(pys[o][t], wf8t[g], GB + g, o, t,
                                      start=False, stop=False)
                        nc.tensor.matmul(
                            pys[o][t][:], bc[:, og:og + 128], uTms[t][:],
                            start=False, stop=True, skip_group_check=True,
                        )
                        yo = ypool.tile([128, 512], BF16, tag="yo",
                                        name=f"yo{ob}_{o}_{t}")
                        if (o + t) % 2 == 0:
                            nc.vector.tensor_copy(yo[:], pys[o][t][:])
                            nc.sync.dma_start(
                                yT_d[og:og + 128, t * 512:(t + 1) * 512], yo[:])
                        else:
                            nc.scalar.copy(yo[:], pys[o][t][:])
                            nc.scalar.dma_start(
                                yT_d[og:og + 128, t * 512:(t + 1) * 512], yo[:])

    nc.compile()
    return nc


def _get_nc():
    global _CACHED_NC
    if _CACHED_NC is None:
        _CACHED_NC = _build()
    return _CACHED_NC


def _prep_in_maps(x, weight, lora_A, lora_B, token_to_slot):
    x = np.asarray(x, dtype=np.float32)
    weight = np.asarray(weight, dtype=np.float32)
    lora_A = np.asarray(lora_A, dtype=np.float32)
    lora_B = np.asarray(lora_B, dtype=np.float32)
    slots = np.asarray(token_to_slot)

    bf = ml_dtypes.bfloat16
    e4 = ml_dtypes.float8_e4m3
    wTf = np.ascontiguousarray(weight.T)                                   # [D_IN, D_OUT] f32
    wT = wTf.astype(bf)
    aTf = np.ascontiguousarray(lora_A.transpose(2, 0, 1).reshape(D_IN, LR))  # [D_IN, L*R] f32
    bC = np.ascontiguousarray(lora_B.transpose(0, 2, 1).reshape(LR, D_OUT)).astype(bf) # [L*R, D_OUT]

    # fp8 pair-packed W tail (k rows KB*128..D_IN), scaled x8:
    # wf8[g*128+p, ob*1024 + i*512 + oc] = W^T[KB*128 + g*256 + i*128 + p, ob*512+oc] * 8
    wf8 = np.ascontiguousarray(
        wTf[KB * 128:].reshape(GT - GB, 2, 128, OB, 512).transpose(0, 2, 3, 1, 4)
        .reshape((GT - GB) * 128, OB * 1024) * 8.0).astype(e4)
    # af8[g*128+p, i*512 + r] = aT[g*256 + i*128 + p, r] * 8  (r < 128; rest 0)
    # af8[p, g*256 + i*128 + r] = aT[g*256 + i*128 + p, r] * 8  (compact)
    af8 = np.ascontiguousarray(
        (aTf.reshape(GT, 2, 128, LR).transpose(2, 0, 1, 3) * 8.0)
        .reshape(128, GT * 2 * LR)).astype(e4)

    # One-hot mask over stacked adapter rows; out-of-range slots -> all-zero.
    maskT = np.zeros((LR, T), dtype=np.float32)
    for l in range(L):
        maskT[l * R:(l + 1) * R, :] = (slots == l).astype(np.float32)[None, :]
    maskT = maskT.astype(bf)

    in_maps = []
    for c in range(N_CORES):
        tsl = slice(c * T_SH, (c + 1) * T_SH)
        xTc = np.ascontiguousarray(x[tsl, :].T)                            # [D_IN, T_SH] f32
        # xf8[g*128+p, t*1024 + i*512 + c] = x^T[g*256 + i*128 + p, t*512+c] / 8
        # xf8[p, g*2048 + t*1024 + i*512 + c] = x^T[g*256+i*128+p, t*512+c]/8
        xf8 = np.ascontiguousarray(
            xTc.reshape(GT, 2, 128, NT, 512).transpose(2, 0, 3, 1, 4)
            .reshape(128, GT * NT * 1024) / 8.0).astype(e4)
        in_maps.append({
            "xT": xTc[:KB * 128].astype(bf),
            "wT": wT,
            "xf8": xf8,
            "wf8": wf8,
            "af8": af8,
            "bC": bC,
            "maskT": np.ascontiguousarray(maskT[:, tsl]),
        })
    return in_maps


def _run(inputs, trace=False, trace_cores=None):
    nc = _get_nc()
    in_maps = _prep_in_maps(**inputs)
    res = run_bass_kernel_spmd(
        nc, in_maps, core_ids=list(range(N_CORES)),
        trace=trace, trace_cores=trace_cores,
    )
    y = np.concatenate([res.results[c]["yT"].astype(np.float32).T
                        for c in range(N_CORES)], axis=0)
    y = np.ascontiguousarray(y)
    return y, res


def _validate(inputs, y):
    """Cheap host-side sanity check: project y onto a random vector and compare
    with the host-computed projection. Catches the (rare, transient) device
    corruption observed on this setup; costs <1 s on host BLAS."""
    x = np.asarray(inputs["x"], dtype=np.float32)
    weight = np.asarray(inputs["weight"], dtype=np.float32)
    lora_A = np.asarray(inputs["lora_A"], dtype=np.float32)
    lora_B = np.asarray(inputs["lora_B"], dtype=np.float32)
    slots = np.asarray(inputs["token_to_slot"])

    rng = np.random.default_rng(12345)
    r = rng.standard_normal(D_OUT).astype(np.float64)

    base = x.astype(np.float64) @ (weight.astype(np.float64).T @ r)      # [T]
    aT = lora_A.transpose(2, 0, 1).reshape(D_IN, LR)                      # [D_IN, LR]
    bC = lora_B.transpose(0, 2, 1).reshape(LR, D_OUT)                     # [LR, D_OUT]
    u = (x @ aT).astype(np.float64)                                       # [T, LR]
    m = np.zeros((T, LR))
    for l in range(L):
        m[:, l * R:(l + 1) * R] = (slots == l).astype(np.float64)[:, None]
    exp = base + (u * m) @ (bC.astype(np.float64) @ r)                    # [T]
    got = y.astype(np.float64) @ r
    scale = np.abs(exp).max()
    rel = np.abs(got - exp).max() / scale
    # bf16 + fp8-tail operand quantization puts the projection error in the
    # low 1e-3s; the corruption this guards against is much larger.
    return rel < 5e-2


def kernel(x, weight, lora_A, lora_B, token_to_slot):
    inputs = dict(x=x, weight=weight, lora_A=lora_A, lora_B=lora_B,
                  token_to_slot=token_to_slot)
    y = None
    for _attempt in range(3):
        y, _ = _run(inputs)
        if _validate(inputs, y):
            break
    return y
